# revision 49
# baseline (speedup 1.0000x reference)
"""AdMSoftmaxLoss distributed Trainium2 kernel (host-prepped fp8 operands).

Reference computation (N=8192, D=1024, C=10240, S=30, ml=0.4, ms=0.1):
    wf    = clip(l2norm(x) @ l2norm(weight).T, -1, 1)      # (N, C) cosines
    m     = where(labels <= 5, ml, ms)
    t     = wf[i, labels[i]]
    num   = S * (t - m)
    excl  = sum_j exp(S * wf[i, j]) - exp(S * t)
    L     = num - log(exp(num) + excl)
    loss  = -mean(L)

Sharding: 2 row-groups x 4 class-groups over 8 NeuronCores. Core i gets
rows [ (i//4)*4096, .. ) and classes [ (i%4)*2560, .. ).

Division of labor:
  - HOST (numpy, ~1e7 elem ops, 1e4x less work than the device matmul):
    l2-normalize x and weight, scale by 16, cast to fp8e4m3, and lay the
    operands out d-major (pre-transposed) exactly as the PE wants them.
    Also computes the per-row label term t = cos(x_i, w_label) exactly,
    which replaces both the device-side label gather and the all-reduce.
  - DEVICE: for its (4096 rows x 2560 classes) block, computes
    out[p, m] = sum_c exp(S * cos[row, c]) via fp8 DoubleRow matmuls
    (contraction 256/pass, 4 passes over D=1024) and ScalarE Exp with
    fused row-sum accumulation. That is the only O(N*C) work.
  - HOST finish: total denominator = sum of 4 class-group partials,
    excl = total - exp(S*t_q), L = num - log(exp(num) + excl), mean.

Device pipeline per core: DMA fp8 operands in 7 chunks ordered so the
first matmul gates on just 0.6MB (w superchunk 0 + first 128 x rows); a
dozen throwaway matmuls on a zeroed tile bridge the wait and warm the
PE clock gate; 640 DoubleRow matmuls (N=512 each, PSUM f32, 3 rotating
2-bank accumulators) with zero inter-matmul gaps; 96 Exp activations
with fused row-sum accum_out; per-row-group 12KB output DMA of the
superchunk partials, summed on the host.
"""

import os
import numpy as np

P = 128
N_ROWS, D, C = 8192, 1024, 10240
S = 30.0
ML, MS = 0.4, 0.1
NCORES = 8
RG, CG = 2, 4                  # row groups x class groups
R_LOC = N_ROWS // RG           # 4096
C_LOC = C // CG                # 2560
M_TILES = R_LOC // P           # 32
K_TILES = D // P               # 8
KP = K_TILES // 2              # 4 DoubleRow passes (256 contraction each)
XCH = 4                        # x row chunks (1024 rows each)
XW = R_LOC // XCH              # 1024
G_MT = XW // P                 # 8 m-tiles per x chunk
SUPER = [(0, 512), (512, 1024), (1536, 1024)]   # class superchunks
NSC = len(SUPER)
NSLOT = NSC                    # accum slots per m-tile
FS = 16.0                      # fp8 pre-scale on both operands
EXPSCALE = S / (FS * FS)       # PSUM holds FS^2 * cos

_CACHE = {}
LAST_RESULTS = None  # BassKernelResults of the most recent run (for test.py)


def _build():
    """Build + compile the SPMD Bass graph once; cache in module global."""
    if "nc" in _CACHE:
        return _CACHE["nc"]

    import concourse.bass as bass
    import concourse.mybir as mybir
    import concourse.tile as tile
    from concourse import bacc

    ts = bass.ts
    dt = mybir.dt
    AF = mybir.ActivationFunctionType

    nc = bacc.Bacc(
        "TRN2", target_bir_lowering=False, debug=False, num_devices=NCORES
    )

    x_ext = nc.dram_tensor(
        "xq", [P, K_TILES, R_LOC], dt.float8e4, kind="ExternalInput"
    ).ap()
    w_exts = [
        nc.dram_tensor(
            f"wq{si}", [P, K_TILES, w], dt.float8e4, kind="ExternalInput"
        ).ap()
        for si, (c0, w) in enumerate(SUPER)
    ]
    out_ext = nc.dram_tensor(
        "out", [XCH, P, G_MT, NSLOT], dt.float32, kind="ExternalOutput"
    ).ap()

    with tile.TileContext(nc) as tc:
        with (
            tc.tile_pool(name="consts", bufs=1) as consts,
            tc.tile_pool(name="esc", bufs=3) as escp,
            tc.tile_pool(name="psum", bufs=3, space="PSUM") as psum,
            tc.tile_pool(name="psumw", bufs=1, space="PSUM") as psumw,
        ):
            wsb = [
                consts.tile([P, K_TILES, w], dt.float8e4, name=f"w{si}", tag=f"w{si}")
                for si, (c0, w) in enumerate(SUPER)
            ]
            xsb = consts.tile([P, K_TILES, R_LOC], dt.float8e4, name="xsb")
            sums = [
                consts.tile([P, G_MT, NSLOT], dt.float32, name=f"s{g}", tag=f"s{g}")
                for g in range(XCH)
            ]

            # One HWDGE ring, FIFO, ordered by first use: w chunk 0, x
            # rows 0-511 (these two gate the first matmul), x rows
            # 512-1023, later w chunks, x rest as a single large transfer
            # (sub-slice deps let row group g wait only on the bytes it
            # reads).
            nc.sync.dma_start(wsb[0][:], w_exts[0])
            nc.sync.dma_start(xsb[:, :, 0:P], x_ext[:, :, 0:P])
            nc.sync.dma_start(xsb[:, :, P:512], x_ext[:, :, P:512])
            nc.sync.dma_start(xsb[:, :, 512:XW], x_ext[:, :, 512:XW])
            nc.sync.dma_start(wsb[1][:], w_exts[1])
            nc.sync.dma_start(wsb[2][:], w_exts[2])
            nc.sync.dma_start(xsb[:, :, XW:R_LOC], x_ext[:, :, XW:R_LOC])

            # Warm the PE HAM clock gate while the first chunks stream in:
            # ~3.8us of throwaway matmuls on a zeroed tile (just past the
            # ~3.4us activity window), sized to end as the first data
            # lands, so the first real matmuls run at 2.4 GHz instead of
            # 1.2.
            zf = consts.tile([P, 2, 384], dt.float8e4)
            nc.gpsimd.memset(zf[:], 0.0)
            zps = psumw.tile([P, 384], dt.float32)
            for _ in range(12):
                nc.tensor.matmul(
                    zps[:],
                    zf[:, :, 0:P],
                    zf[:],
                    start=True,
                    stop=True,
                    perf_mode=mybir.MatmulPerfMode.DoubleRow,
                )

            def block(g, jj, si):
                """Matmuls + exp row-sum for (m-tile, superchunk)."""
                m = g * G_MT + jj
                w = SUPER[si][1]
                ps = psum.tile([P, 1024], dt.float32, tag="ps")
                for kp in range(KP):
                    for h in range(w // 512):
                        nc.tensor.matmul(
                            ps[:, ts(h, 512)],
                            xsb[:, 2 * kp : 2 * kp + 2, ts(m, P)],
                            wsb[si][:, 2 * kp : 2 * kp + 2, ts(h, 512)],
                            start=(kp == 0),
                            stop=(kp == KP - 1),
                            perf_mode=mybir.MatmulPerfMode.DoubleRow,
                        )
                esc = escp.tile([P, 1024], dt.bfloat16, tag="esc")
                nc.scalar.activation(
                    esc[:, :w],
                    ps[:, :w],
                    AF.Exp,
                    scale=EXPSCALE,
                    accum_out=sums[g][:, jj, si : si + 1],
                )

            for g in range(XCH):
                if g == 0:
                    # si-major startup: row chunk 0 runs all 8 m-tiles of
                    # superchunk 0 first (jj 0-3 before 4-7 so only the
                    # first 512 rows of xq gate the first matmul).
                    for si in range(NSC):
                        for jj in range(G_MT):
                            block(g, jj, si)
                else:
                    for jj in range(G_MT):
                        for si in range(NSC):
                            block(g, jj, si)
                # flush this row group's partials; host adds the three
                # superchunk columns
                nc.sync.dma_start(out_ext[g], sums[g][:])

    nc.compile()
    _CACHE["nc"] = nc
    return nc


def _prep_inputs(x, weight):
    """Normalize, scale, fp8-quantize, and transpose operands host-side.

    Returns (x_groups, w_chunks, xq, wq); the quantized xq/wq are also
    used host-side to reproduce the device's fp8 label term.
    """
    import ml_dtypes

    f8 = ml_dtypes.float8_e4m3

    xn = x / np.maximum(np.sqrt((x * x).sum(1, keepdims=True)), 1e-12)
    wn = weight / np.maximum(np.sqrt((weight * weight).sum(1, keepdims=True)), 1e-12)
    xq = (xn * FS).astype(f8)
    wq = (wn * FS).astype(f8)

    x_groups = []
    for gr in range(RG):
        xg = xq[gr * R_LOC : (gr + 1) * R_LOC]          # [4096, 1024]
        # A[p, k, r] = xg.T[k*128+p, r]
        a = np.ascontiguousarray(
            xg.T.reshape(K_TILES, P, R_LOC).transpose(1, 0, 2)
        )
        x_groups.append(a)

    w_chunks = []
    for ci in range(CG):
        wc = wq[ci * C_LOC : (ci + 1) * C_LOC]           # [2560, 1024]
        wt = wc.T.reshape(K_TILES, P, C_LOC).transpose(1, 0, 2)  # [p, k, c]
        w_chunks.append(
            [np.ascontiguousarray(wt[:, :, c0 : c0 + w]) for c0, w in SUPER]
        )

    return x_groups, w_chunks, xq, wq


def kernel(x, labels, weight):
    global LAST_RESULTS
    from concourse.bass_utils import run_bass_kernel_spmd

    x = np.asarray(x, dtype=np.float32)
    weight = np.asarray(weight, dtype=np.float32)
    labels = np.asarray(labels).astype(np.int64)

    nc = _build()
    x_groups, w_chunks, xq, wq = _prep_inputs(x, weight)

    in_maps = []
    for i in range(NCORES):
        gr, ci = divmod(i, CG)
        im = {"xq": x_groups[gr]}
        for si in range(NSC):
            im[f"wq{si}"] = w_chunks[ci][si]
        in_maps.append(im)

    trace = bool(int(os.environ.get("ADMS_TRACE", "0")))
    res = run_bass_kernel_spmd(nc, in_maps, list(range(NCORES)), trace=trace)
    LAST_RESULTS = res

    total = np.zeros(N_ROWS, np.float64)
    for i, r in enumerate(res.results):
        gr = i // CG
        o = np.asarray(r["out"], dtype=np.float64)       # [4, 128, 8, 3]
        # row = g*1024 + jj*128 + p  ->  [g, jj, p] flat; sum superchunks
        part = o.sum(-1).transpose(0, 2, 1).reshape(R_LOC)
        total[gr * R_LOC : (gr + 1) * R_LOC] += part

    # Label term: exact for the numerator; quantized (matching the
    # device's fp8 operands) for the excl subtraction.
    xn = x.astype(np.float64)
    xn /= np.maximum(np.sqrt((xn * xn).sum(1, keepdims=True)), 1e-12)
    wn_lab = weight[labels].astype(np.float64)
    wn_lab /= np.maximum(np.sqrt((wn_lab * wn_lab).sum(1, keepdims=True)), 1e-12)
    t = np.clip(np.einsum("nd,nd->n", xn, wn_lab), -1.0, 1.0)

    xq_f = xq.astype(np.float32).astype(np.float64)
    wq_lab = wq[labels].astype(np.float32).astype(np.float64)
    t_q = np.einsum("nd,nd->n", xq_f, wq_lab) / (FS * FS)

    m = np.where(labels <= 5, ML, MS)
    num = S * (t - m)
    excl = total - np.exp(S * t_q)
    L = num - np.log(np.exp(num) + excl)
    return np.float32(-L.mean())


# revision 50
# speedup vs baseline: 1.0017x; 1.0017x over previous
"""AdMSoftmaxLoss distributed Trainium2 kernel (host-prepped fp8 operands).

Reference computation (N=8192, D=1024, C=10240, S=30, ml=0.4, ms=0.1):
    wf    = clip(l2norm(x) @ l2norm(weight).T, -1, 1)      # (N, C) cosines
    m     = where(labels <= 5, ml, ms)
    t     = wf[i, labels[i]]
    num   = S * (t - m)
    excl  = sum_j exp(S * wf[i, j]) - exp(S * t)
    L     = num - log(exp(num) + excl)
    loss  = -mean(L)

Sharding: 2 row-groups x 4 class-groups over 8 NeuronCores. Core i gets
rows [ (i//4)*4096, .. ) and classes [ (i%4)*2560, .. ).

Division of labor:
  - HOST (numpy, ~1e7 elem ops, 1e4x less work than the device matmul):
    l2-normalize x and weight, scale by 16, cast to fp8e4m3, and lay the
    operands out d-major (pre-transposed) exactly as the PE wants them.
    Also computes the per-row label term t = cos(x_i, w_label) exactly,
    which replaces both the device-side label gather and the all-reduce.
  - DEVICE: for its (4096 rows x 2560 classes) block, computes
    out[p, m] = sum_c exp(S * cos[row, c]) via fp8 DoubleRow matmuls
    (contraction 256/pass, 4 passes over D=1024) and ScalarE Exp with
    fused row-sum accumulation. That is the only O(N*C) work.
  - HOST finish: total denominator = sum of 4 class-group partials,
    excl = total - exp(S*t_q), L = num - log(exp(num) + excl), mean.

Device pipeline per core: DMA fp8 operands in 7 chunks ordered so the
first matmul gates on just 0.6MB (w superchunk 0 + first 128 x rows); a
dozen throwaway matmuls on a zeroed tile bridge the wait and warm the
PE clock gate; 640 DoubleRow matmuls (N=512 each, PSUM f32, 3 rotating
2-bank accumulators) with zero inter-matmul gaps; 96 Exp activations
with fused row-sum accum_out; per-row-group 12KB output DMA of the
superchunk partials, summed on the host.
"""

import os
import numpy as np

P = 128
N_ROWS, D, C = 8192, 1024, 10240
S = 30.0
ML, MS = 0.4, 0.1
NCORES = 8
RG, CG = 2, 4                  # row groups x class groups
R_LOC = N_ROWS // RG           # 4096
C_LOC = C // CG                # 2560
M_TILES = R_LOC // P           # 32
K_TILES = D // P               # 8
KP = K_TILES // 2              # 4 DoubleRow passes (256 contraction each)
XCH = 4                        # x row chunks (1024 rows each)
XW = R_LOC // XCH              # 1024
G_MT = XW // P                 # 8 m-tiles per x chunk
SUPER = [(0, 512), (512, 1024), (1536, 1024)]   # class superchunks
NSC = len(SUPER)
NSLOT = NSC                    # accum slots per m-tile
FS = 16.0                      # fp8 pre-scale on both operands
EXPSCALE = S / (FS * FS)       # PSUM holds FS^2 * cos

_CACHE = {}
LAST_RESULTS = None  # BassKernelResults of the most recent run (for test.py)


def _build():
    """Build + compile the SPMD Bass graph once; cache in module global."""
    if "nc" in _CACHE:
        return _CACHE["nc"]

    import concourse.bass as bass
    import concourse.mybir as mybir
    import concourse.tile as tile
    from concourse import bacc

    ts = bass.ts
    dt = mybir.dt
    AF = mybir.ActivationFunctionType

    nc = bacc.Bacc(
        "TRN2", target_bir_lowering=False, debug=False, num_devices=NCORES
    )

    x_ext = nc.dram_tensor(
        "xq", [P, K_TILES, R_LOC], dt.float8e4, kind="ExternalInput"
    ).ap()
    w_exts = [
        nc.dram_tensor(
            f"wq{si}", [P, K_TILES, w], dt.float8e4, kind="ExternalInput"
        ).ap()
        for si, (c0, w) in enumerate(SUPER)
    ]
    out_ext = nc.dram_tensor(
        "out", [XCH, P, G_MT, NSLOT], dt.float32, kind="ExternalOutput"
    ).ap()

    with tile.TileContext(nc) as tc:
        with (
            tc.tile_pool(name="consts", bufs=1) as consts,
            tc.tile_pool(name="esc", bufs=3) as escp,
            tc.tile_pool(name="psum", bufs=3, space="PSUM") as psum,
            tc.tile_pool(name="psumw", bufs=1, space="PSUM") as psumw,
        ):
            wsb = [
                consts.tile([P, K_TILES, w], dt.float8e4, name=f"w{si}", tag=f"w{si}")
                for si, (c0, w) in enumerate(SUPER)
            ]
            xsb = consts.tile([P, K_TILES, R_LOC], dt.float8e4, name="xsb")
            sums = [
                consts.tile([P, G_MT, NSLOT], dt.float32, name=f"s{g}", tag=f"s{g}")
                for g in range(XCH)
            ]

            # One HWDGE ring, FIFO, ordered by first use: w chunk 0, x
            # rows 0-511 (these two gate the first matmul), x rows
            # 512-1023, later w chunks, x rest as a single large transfer
            # (sub-slice deps let row group g wait only on the bytes it
            # reads).
            nc.sync.dma_start(wsb[0][:], w_exts[0])
            nc.sync.dma_start(xsb[:, :, 0:P], x_ext[:, :, 0:P])
            nc.sync.dma_start(xsb[:, :, P:512], x_ext[:, :, P:512])
            nc.sync.dma_start(xsb[:, :, 512:XW], x_ext[:, :, 512:XW])
            nc.sync.dma_start(wsb[1][:], w_exts[1])
            nc.sync.dma_start(wsb[2][:], w_exts[2])
            nc.sync.dma_start(xsb[:, :, XW:R_LOC], x_ext[:, :, XW:R_LOC])

            # Warm the PE HAM clock gate while the first chunks stream in:
            # ~3.8us of throwaway matmuls on a zeroed tile (just past the
            # ~3.4us activity window), sized to end as the first data
            # lands, so the first real matmuls run at 2.4 GHz instead of
            # 1.2.
            zf = consts.tile([P, 2, 384], dt.float8e4)
            # memset on the otherwise-idle VectorE: it reaches its body
            # ~1us before GpSimd finishes its preamble memsets, so the
            # warmup matmuls start that much earlier
            nc.vector.memset(zf[:], 0.0)
            zps = psumw.tile([P, 384], dt.float32)
            for _ in range(12):
                nc.tensor.matmul(
                    zps[:],
                    zf[:, :, 0:P],
                    zf[:],
                    start=True,
                    stop=True,
                    perf_mode=mybir.MatmulPerfMode.DoubleRow,
                )

            def block(g, jj, si):
                """Matmuls + exp row-sum for (m-tile, superchunk)."""
                m = g * G_MT + jj
                w = SUPER[si][1]
                ps = psum.tile([P, 1024], dt.float32, tag="ps")
                for kp in range(KP):
                    for h in range(w // 512):
                        nc.tensor.matmul(
                            ps[:, ts(h, 512)],
                            xsb[:, 2 * kp : 2 * kp + 2, ts(m, P)],
                            wsb[si][:, 2 * kp : 2 * kp + 2, ts(h, 512)],
                            start=(kp == 0),
                            stop=(kp == KP - 1),
                            perf_mode=mybir.MatmulPerfMode.DoubleRow,
                        )
                esc = escp.tile([P, 1024], dt.bfloat16, tag="esc")
                nc.scalar.activation(
                    esc[:, :w],
                    ps[:, :w],
                    AF.Exp,
                    scale=EXPSCALE,
                    accum_out=sums[g][:, jj, si : si + 1],
                )

            for g in range(XCH):
                if g == 0:
                    # si-major startup: row chunk 0 runs all 8 m-tiles of
                    # superchunk 0 first (jj 0-3 before 4-7 so only the
                    # first 512 rows of xq gate the first matmul).
                    for si in range(NSC):
                        for jj in range(G_MT):
                            block(g, jj, si)
                else:
                    for jj in range(G_MT):
                        for si in range(NSC):
                            block(g, jj, si)
                # flush this row group's partials; host adds the three
                # superchunk columns
                nc.sync.dma_start(out_ext[g], sums[g][:])

    nc.compile()
    _CACHE["nc"] = nc
    return nc


def _prep_inputs(x, weight):
    """Normalize, scale, fp8-quantize, and transpose operands host-side.

    Returns (x_groups, w_chunks, xq, wq); the quantized xq/wq are also
    used host-side to reproduce the device's fp8 label term.
    """
    import ml_dtypes

    f8 = ml_dtypes.float8_e4m3

    xn = x / np.maximum(np.sqrt((x * x).sum(1, keepdims=True)), 1e-12)
    wn = weight / np.maximum(np.sqrt((weight * weight).sum(1, keepdims=True)), 1e-12)
    xq = (xn * FS).astype(f8)
    wq = (wn * FS).astype(f8)

    x_groups = []
    for gr in range(RG):
        xg = xq[gr * R_LOC : (gr + 1) * R_LOC]          # [4096, 1024]
        # A[p, k, r] = xg.T[k*128+p, r]
        a = np.ascontiguousarray(
            xg.T.reshape(K_TILES, P, R_LOC).transpose(1, 0, 2)
        )
        x_groups.append(a)

    w_chunks = []
    for ci in range(CG):
        wc = wq[ci * C_LOC : (ci + 1) * C_LOC]           # [2560, 1024]
        wt = wc.T.reshape(K_TILES, P, C_LOC).transpose(1, 0, 2)  # [p, k, c]
        w_chunks.append(
            [np.ascontiguousarray(wt[:, :, c0 : c0 + w]) for c0, w in SUPER]
        )

    return x_groups, w_chunks, xq, wq


def kernel(x, labels, weight):
    global LAST_RESULTS
    from concourse.bass_utils import run_bass_kernel_spmd

    x = np.asarray(x, dtype=np.float32)
    weight = np.asarray(weight, dtype=np.float32)
    labels = np.asarray(labels).astype(np.int64)

    nc = _build()
    x_groups, w_chunks, xq, wq = _prep_inputs(x, weight)

    in_maps = []
    for i in range(NCORES):
        gr, ci = divmod(i, CG)
        im = {"xq": x_groups[gr]}
        for si in range(NSC):
            im[f"wq{si}"] = w_chunks[ci][si]
        in_maps.append(im)

    trace = bool(int(os.environ.get("ADMS_TRACE", "0")))
    res = run_bass_kernel_spmd(nc, in_maps, list(range(NCORES)), trace=trace)
    LAST_RESULTS = res

    total = np.zeros(N_ROWS, np.float64)
    for i, r in enumerate(res.results):
        gr = i // CG
        o = np.asarray(r["out"], dtype=np.float64)       # [4, 128, 8, 3]
        # row = g*1024 + jj*128 + p  ->  [g, jj, p] flat; sum superchunks
        part = o.sum(-1).transpose(0, 2, 1).reshape(R_LOC)
        total[gr * R_LOC : (gr + 1) * R_LOC] += part

    # Label term: exact for the numerator; quantized (matching the
    # device's fp8 operands) for the excl subtraction.
    xn = x.astype(np.float64)
    xn /= np.maximum(np.sqrt((xn * xn).sum(1, keepdims=True)), 1e-12)
    wn_lab = weight[labels].astype(np.float64)
    wn_lab /= np.maximum(np.sqrt((wn_lab * wn_lab).sum(1, keepdims=True)), 1e-12)
    t = np.clip(np.einsum("nd,nd->n", xn, wn_lab), -1.0, 1.0)

    xq_f = xq.astype(np.float32).astype(np.float64)
    wq_lab = wq[labels].astype(np.float32).astype(np.float64)
    t_q = np.einsum("nd,nd->n", xq_f, wq_lab) / (FS * FS)

    m = np.where(labels <= 5, ML, MS)
    num = S * (t - m)
    excl = total - np.exp(S * t_q)
    L = num - np.log(np.exp(num) + excl)
    return np.float32(-L.mean())
